# revision 9
# baseline (speedup 1.0000x reference)
"""
Trainium2 Bass kernel for nn_Attention_335007449901 (sparse window attention).

Model (per image, eval mode):
  q = BN(conv1x1(x, wq)); k = BN(conv1x1(x, wk)); v = BN(conv1x1(x, wv))
  7x7 windows over the 112x112 image -> T=256 window tokens, token
  features = (channel, within-window position p) pairs.
  dots[i,j] = <q_i, k_j> * 0.125 ; attn = softmax_j ; out = attn @ v
  y = gelu(out); z = BN(conv1x1(y, wo) + bo); out = gelu(z + x)

Sharding: pure data parallel over batch, 4 images per core on 8 cores.

Key structure (vs the v1 kernel this replaces):
  * Host stages x twice: channel-major window-permuted bf16 (x_win, feeds
    the u-conv/dots moving side + the residual) and token-major fp8e4
    (xT8, [j_local, (jc, p, ch)], feeds the attention-average as
    DoubleRow stationary).  Output returns in bf16 window layout and is
    un-permuted + upcast on the host.
  * q/k fold into M = wk_f^T wq_f (x256 scale; softmax exp applies 2^-8).
    dots_T[j,i] = sum_p (M^T x_p)^T x_p stays bf16 (fp8 there fails the
    error budget).  The q-bias term c[j] is a bf16 fixed-weight run whose
    49 matmuls double as PE fillers between u-conv PSUM-cast waits.
  * The attention average is applied to x (not v): avx_p = A @ x_p^T via
    fp8 DoubleRow matmuls (contraction j=256 = 2 k-tiles), attn quantized
    to e4m3 scaled x32.  Then v' = wv_f avx (bf16), g = gelu(v'/32 + Bv)
    written as fp8, and the out conv is an fp8 DoubleRow over HIDE_C=256
    with wo scaled x8.  ~2x fewer PE cycles than conv-then-average bf16.
  * PSUM (8 banks): mega pool 3x[128,1024] (u/avx/v/o rotate, 6 banks) +
    dots [128,512] (1) + small (c_ps/s_ps, 1).
  * Engine split: PE matmuls; ACT exp + both gelus on [128,1024] tiles
    (act-table swaps hidden via dummy ops); DVE avx-cast + fused residual
    (o/8 + x) + attn quant; GPSIMD the u-conv PSUM->bf16 casts.
"""

import numpy as np

IN_C = 128
HIDE_C = 256
OUT_C = 128
WS = 7
SCALE = 0.125
EPS = 1e-5
B, H, W = 32, 112, 112
HW = H * W          # 12544
H1 = H // WS        # 16
W1 = W // WS        # 16
T = H1 * W1         # 256 windows
NP = WS * WS        # 49 positions
NCORES = 8
BPC = B // NCORES   # images per core

SM = 256.0          # M / h scale (exp applies 1/SM)
SA = 32.0           # attn fp8 scale (folded out in gelu1's scale)
SO = 8.0            # wo fp8 scale (folded out in the residual stt)

F32 = np.float32


def build_bass_kernel(bpc=BPC):
    import concourse.bass as bass
    import concourse.tile as tile
    import concourse.mybir as mybir
    from concourse import bacc

    f32 = mybir.dt.float32
    bf16 = mybir.dt.bfloat16
    fp8 = mybir.dt.float8e4
    AF = mybir.ActivationFunctionType
    DR = mybir.MatmulPerfMode.DoubleRow
    ALU = mybir.AluOpType

    nc = bacc.Bacc("TRN2", target_bir_lowering=False)

    xw_d = nc.dram_tensor("x_win", [bpc, IN_C, NP * T], bf16,
                          kind="ExternalInput")
    xt_d = nc.dram_tensor("xT8", [bpc, 128, 2 * NP * 128], fp8,
                          kind="ExternalInput")
    m_d = nc.dram_tensor("m", [IN_C, IN_C], bf16, kind="ExternalInput")
    h_d = nc.dram_tensor("hcol", [IN_C, 1], bf16, kind="ExternalInput")
    wvT_d = nc.dram_tensor("wvT", [IN_C, HIDE_C], bf16, kind="ExternalInput")
    wo_d = nc.dram_tensor("woP", [128, 2 * OUT_C], fp8, kind="ExternalInput")
    # packed per-partition fp32 bias columns: [Bv_lo, Bv_hi, Bo]
    bias_d = nc.dram_tensor("biases", [128, 3], f32, kind="ExternalInput")
    out_d = nc.dram_tensor("out", [bpc, OUT_C, NP * T], bf16,
                           kind="ExternalOutput")

    # 49 positions -> 12 chunks of 4 + final single
    CHUNKS = [(k * 4, 4) for k in range(12)] + [(48, 1)]

    with tile.TileContext(nc) as tc:
        with (
            tc.tile_pool(name="singles", bufs=1) as singles,
            tc.tile_pool(name="xwin", bufs=2) as xw_pool,
            tc.tile_pool(name="xt8", bufs=2) as xt_pool,
            tc.tile_pool(name="u_sb", bufs=4) as u_pool,
            tc.tile_pool(name="attn", bufs=2) as attn_pool,
            tc.tile_pool(name="avx_sb", bufs=3) as avx_pool,
            tc.tile_pool(name="g_sb", bufs=2) as g_pool,
            tc.tile_pool(name="tmp_sb", bufs=3) as tmp_pool,
            tc.tile_pool(name="out_sb", bufs=3) as out_pool,
            tc.tile_pool(name="small_sb", bufs=2) as small_pool,
            tc.tile_pool(name="ps_mega", bufs=3, space="PSUM") as ps_mega,
            tc.tile_pool(name="ps_dots", bufs=1, space="PSUM") as ps_dots,
            tc.tile_pool(name="ps_small", bufs=1, space="PSUM") as ps_small,
        ):
            # ---- weights / constants (resident) ----
            m_sb = singles.tile([128, IN_C], bf16)
            nc.sync.dma_start(out=m_sb, in_=m_d.ap())
            h_sb = singles.tile([128, 1], bf16)
            nc.sync.dma_start(out=h_sb, in_=h_d.ap())
            wvT = singles.tile([128, HIDE_C], bf16)
            nc.sync.dma_start(out=wvT, in_=wvT_d.ap())
            woP = singles.tile([128, 2, OUT_C], fp8)
            nc.sync.dma_start(
                out=woP, in_=wo_d.ap().rearrange("p (k o) -> p k o", k=2))
            biases = singles.tile([128, 3], f32)
            nc.sync.dma_start(out=biases, in_=bias_d.ap())
            bv_ap = [biases[:, 0:1], biases[:, 1:2]]
            bo_ap = biases[:, 2:3]

            ones_mat = singles.tile([128, 128], bf16)
            nc.vector.memset(ones_mat, 1.0)
            ones_row = singles.tile([1, T], bf16)
            nc.vector.memset(ones_row, 1.0)

            for img in range(bpc):
                # ---- image loads (window-permuted on the host) ----
                x_win = xw_pool.tile([128, NP * T], bf16, tag="xw")
                for dc in range(4):
                    s = dc * (NP * T // 4)
                    e = s + NP * T // 4
                    nc.sync.dma_start(out=x_win[:, s:e],
                                      in_=xw_d.ap()[img, :, s:e])
                xT8 = xt_pool.tile([128, 2, NP, 128], fp8, tag="xt")
                xt_src = xt_d.ap()[img].rearrange(
                    "p (k q c) -> p k q c", k=2, q=NP)
                for dc in range(2):
                    nc.sync.dma_start(out=xT8[:, dc], in_=xt_src[:, dc])

                # dummy exp: pulls the exp ACT-table load off the critical
                # chain (runs while PE does phase 1)
                scratch = small_pool.tile([128, 1], f32, tag="scr")
                nc.scalar.activation(scratch, biases[:, 0:1], AF.Exp)

                # ---- phase 1: dots_T (bf16) + c (fixed-weight filler) ----
                dots_t = ps_dots.tile([128, 512], f32, tag="dots",
                                      name="dots")
                dots = [dots_t[:, 0:T], dots_t[:, T:2 * T]]
                c_ps = ps_small.tile([1, T], f32, tag="psm", name="cps")

                def u_conv(ci, p0, npos):
                    N = npos * T
                    u_ps = ps_mega.tile([128, 1024], f32, tag="mega")
                    for hh in range(0, N, 512):
                        hn = min(512, N - hh)
                        nc.tensor.matmul(
                            u_ps[:, hh:hh + hn], lhsT=m_sb,
                            rhs=x_win[:, p0 * T + hh:p0 * T + hh + hn],
                            start=True, stop=True)
                    u_sb = u_pool.tile([128, 1024], bf16, tag="u")
                    # gpsimd can't read PSUM; split casts DVE/ACT for balance
                    if ci % 3 == 2:
                        nc.scalar.activation(u_sb[:, :N], u_ps[:, :N],
                                             AF.Copy, scale=1.0)
                    else:
                        nc.vector.tensor_copy(u_sb[:, :N], u_ps[:, :N])
                    return u_sb

                def dots_mms(p0, npos, u_sb, first):
                    for pi in range(npos):
                        for jh in (0, 1):
                            nc.tensor.matmul(
                                dots[jh],
                                lhsT=u_sb[:, pi * T + jh * 128:
                                          pi * T + jh * 128 + 128],
                                rhs=x_win[:, (p0 + pi) * T:(p0 + pi + 1) * T],
                                start=first and pi == 0 and jh == 0,
                                stop=False, skip_group_check=True)

                def c_mms(p0, npos):
                    for pi in range(npos):
                        p = p0 + pi
                        nc.tensor.matmul(c_ps, lhsT=h_sb,
                                         rhs=x_win[:, p * T:(p + 1) * T],
                                         start=p == 0, stop=p == NP - 1,
                                         skip_group_check=True)

                UCH = [(k * 4, 4) for k in range(12)] + [(48, 1)]
                pend = []
                for ci, (p0, npos) in enumerate(UCH):
                    u_sb = u_conv(ci, p0, npos)
                    c_mms(p0, npos)  # PE filler, no dep on u casts
                    if len(pend) >= 2:
                        dots_mms(*pend.pop(0))
                    pend.append((p0, npos, u_sb, ci == 0))
                for a in pend:
                    dots_mms(*a)

                c_sb = small_pool.tile([1, T], bf16, tag="csb")
                nc.vector.tensor_copy(c_sb, c_ps)  # [1,T]: tiny, PSUM src
                for jh in (0, 1):
                    nc.tensor.matmul(
                        dots[jh], lhsT=c_sb[:, jh * 128:jh * 128 + 128],
                        rhs=ones_row, start=False, stop=jh == 1,
                        skip_group_check=True)

                # ---- softmax over j (partition dim of dots_T) ----
                attn_t = attn_pool.tile([128, 512], bf16, tag="at",
                                        name="attn_t")
                nc.scalar.activation(attn_t, dots_t, AF.Exp, scale=1.0 / SM)
                s_ps = ps_small.tile([128, T], f32, tag="psm", name="ssum")
                for jc in (0, 1):
                    nc.tensor.matmul(s_ps, lhsT=ones_mat,
                                     rhs=attn_t[:, jc * T:(jc + 1) * T],
                                     start=jc == 0, stop=jc == 1)
                r_sb = small_pool.tile([128, T], f32, tag="rsb")
                nc.vector.reciprocal(r_sb, s_ps)
                # attn8 = e4m3(SA * attn_t * r), laid out [j_local, jc, i]
                attn8 = attn_pool.tile([128, 2, T], fp8, tag="a8",
                                       name="attn8")
                for jc in (0, 1):
                    nc.vector.scalar_tensor_tensor(
                        attn8[:, jc], in0=attn_t[:, jc * T:(jc + 1) * T],
                        scalar=SA, op0=ALU.mult, in1=r_sb, op1=ALU.mult)

                # dummy gelu: pulls the gelu table load off the chain
                nc.scalar.activation(scratch, biases[:, 0:1], AF.Gelu)

                # ---- phase 2: average-x, v'-conv, gelu, out-conv ----
                for p0, npos in CHUNKS:
                    N = npos * T
                    avx_ps = ps_mega.tile([128, 1024], f32, tag="mega")
                    for pi in range(npos):
                        nc.tensor.matmul(
                            avx_ps[:, pi * T:(pi + 1) * T],
                            lhsT=xT8[:, :, p0 + pi, :], rhs=attn8,
                            start=True, stop=True, perf_mode=DR)
                    avx_sb = avx_pool.tile([128, 1024], bf16, tag="avs")
                    nc.vector.tensor_copy(avx_sb[:, :N], avx_ps[:, :N])

                    g_t = g_pool.tile([128, 2, 1024], fp8, tag="g")
                    for kc in (0, 1):
                        v_ps = ps_mega.tile([128, 1024], f32, tag="mega")
                        for hh in range(0, N, 512):
                            hn = min(512, N - hh)
                            nc.tensor.matmul(
                                v_ps[:, hh:hh + hn],
                                lhsT=wvT[:, kc * 128:(kc + 1) * 128],
                                rhs=avx_sb[:, hh:hh + hn],
                                start=True, stop=True)
                        nc.scalar.activation(g_t[:, kc, :N], v_ps[:, :N],
                                             AF.Gelu, bias=bv_ap[kc],
                                             scale=1.0 / SA)

                    o_ps = ps_mega.tile([128, 1024], f32, tag="mega")
                    for hh in range(0, N, 512):
                        hn = min(512, N - hh)
                        nc.tensor.matmul(o_ps[:, hh:hh + hn], lhsT=woP,
                                         rhs=g_t[:, :, hh:hh + hn],
                                         start=True, stop=True, perf_mode=DR)
                    # tmp = o/SO + x  (residual), then gelu(tmp + Bo)
                    tmp = tmp_pool.tile([128, 1024], bf16, tag="tmp")
                    nc.vector.scalar_tensor_tensor(
                        tmp[:, :N], in0=o_ps[:, :N], scalar=1.0 / SO,
                        op0=ALU.mult, in1=x_win[:, p0 * T:p0 * T + N],
                        op1=ALU.add)
                    o_sb = out_pool.tile([128, 1024], bf16, tag="osb")
                    nc.scalar.activation(o_sb[:, :N], tmp[:, :N], AF.Gelu,
                                         bias=bo_ap, scale=1.0)
                    nc.sync.dma_start(out=out_d.ap()[img, :,
                                                     p0 * T:p0 * T + N],
                                      in_=o_sb[:, :N])

    nc.compile()
    return nc


def fold_params(wq, gq, bq, mq, vq, wk, gk, bk, mk, vk,
                wv, gv, bv, mv, vv, wo, bo, go, bbo, mo, vo):
    """Host-side BN/bias folding. Returns (M, h, wvT, woP, biases)."""
    import ml_dtypes
    bf16 = ml_dtypes.bfloat16
    e4m3 = ml_dtypes.float8_e4m3

    aq = gq / np.sqrt(vq + EPS)
    wq_f = (SCALE * aq)[:, None] * wq
    Bq = SCALE * (bq - aq * mq)

    ak = gk / np.sqrt(vk + EPS)
    wk_f = ak[:, None] * wk          # k bias drops (softmax shift invariance)

    M = (wk_f.T @ wq_f) * SM         # exp() applies 1/SM
    hv = (wk_f.T @ Bq) * SM

    av = gv / np.sqrt(vv + EPS)
    wv_f = av[:, None] * wv
    Bv = bv - av * mv                # applied inside the first gelu

    ao = go / np.sqrt(vo + EPS)
    wo_f = ao[:, None] * wo * SO     # residual stt applies 1/SO
    Bo = ao * (bo - mo) + bbo        # conv bias + BN fold, inside last gelu

    # woP[dlo, k, o] = wo_f[o, k*128 + dlo]  (DoubleRow k-subtile pairs)
    woP = np.ascontiguousarray(
        wo_f.T.reshape(2, 128, OUT_C).transpose(1, 0, 2))

    biases = np.stack([Bv[:128], Bv[128:], Bo], axis=1).astype(F32)
    return (np.ascontiguousarray(M).astype(bf16),
            np.ascontiguousarray(hv[:, None]).astype(bf16),
            np.ascontiguousarray(wv_f.T).astype(bf16),
            np.clip(woP, -240, 240).astype(e4m3),
            biases)


_CACHED = {}


def _get_nc(bpc=BPC):
    if bpc not in _CACHED:
        _CACHED[bpc] = build_bass_kernel(bpc)
    return _CACHED[bpc]


def make_in_maps(inputs):
    import ml_dtypes
    bf16 = ml_dtypes.bfloat16
    e4m3 = ml_dtypes.float8_e4m3

    x = np.asarray(inputs["x"], F32)
    m, hv, wvT, woP, biases = fold_params(
        *[np.asarray(inputs[k], F32) for k in
          ("wq", "gq", "bq", "mq", "vq", "wk", "gk", "bk", "mk", "vk",
           "wv", "gv", "bv", "mv", "vv", "wo", "bo", "go", "bbo", "mo", "vo")]
    )
    # window permute: [B, C, H, W] -> [B, C, p=(ws1 ws2), j=(h1 w1)]
    xwin = x.reshape(B, IN_C, H1, WS, W1, WS).transpose(0, 1, 3, 5, 2, 4)
    xwin = np.ascontiguousarray(xwin.reshape(B, IN_C, NP, T))
    xw_bf = xwin.reshape(B, IN_C, NP * T).astype(bf16)
    # token-major fp8: [B, j_local=128, (jc, p, ch)]
    xt = xwin.transpose(0, 3, 1, 2)          # [B, j, C, p]
    xt = xt.reshape(B, 2, 128, IN_C, NP).transpose(0, 2, 1, 4, 3)
    xt8 = np.clip(xt, -240, 240).astype(e4m3).reshape(B, 128, 2 * NP * 128)

    in_maps = []
    for c in range(NCORES):
        sl = slice(c * BPC, (c + 1) * BPC)
        in_maps.append({"x_win": np.ascontiguousarray(xw_bf[sl]),
                        "xT8": np.ascontiguousarray(xt8[sl]),
                        "m": m, "hcol": hv, "wvT": wvT,
                        "woP": np.ascontiguousarray(
                            woP.reshape(128, 2 * OUT_C)),
                        "biases": biases})
    return in_maps


def kernel(**inputs):
    from concourse.bass_utils import run_bass_kernel_spmd

    in_maps = make_in_maps(inputs)
    nc = _get_nc(BPC)
    res = run_bass_kernel_spmd(nc, in_maps, list(range(NCORES)))
    outs = []
    for c in range(NCORES):
        o = np.asarray(res.results[c]["out"], dtype=F32)
        o = o.reshape(BPC, OUT_C, WS, WS, H1, W1).transpose(0, 1, 4, 2, 5, 3)
        outs.append(o.reshape(BPC, OUT_C, H, W))
    return np.concatenate(outs, axis=0)


# revision 10
# speedup vs baseline: 1.7230x; 1.7230x over previous
"""
Trainium2 Bass kernel for nn_Attention_335007449901 (sparse window attention).

Model (per image, eval mode):
  q = BN(conv1x1(x, wq)); k = BN(conv1x1(x, wk)); v = BN(conv1x1(x, wv))
  7x7 windows over the 112x112 image -> T=256 window tokens, token
  features = (channel, within-window position p) pairs.
  dots[i,j] = <q_i, k_j> * 0.125 ; attn = softmax_j ; out = attn @ v
  y = gelu(out); z = BN(conv1x1(y, wo) + bo); out = gelu(z + x)

Sharding: pure data parallel over batch, 4 images per core on 8 cores.

Key structure:
  * Host stages x twice: channel-major window-permuted bf16 (x_win, feeds
    the u-conv/dots moving side + the residual) and token-major fp8e4
    (xT8, [j_local, (jc, p, ch)], DoubleRow stationary for the attention
    average).  Output returns bf16 window layout; un-permuted on host.
  * q/k fold into M = wk_f^T wq_f (x256; softmax exp applies 2^-8).
    dots_T[j,i] = sum_p (M^T x_p)^T x_p stays bf16 (fp8 fails the error
    budget there).  The q-bias term c[j] is a bf16 fixed-weight run.
  * Attention-average runs on x (not v): avx_p = A @ x_p^T via fp8
    DoubleRow (contraction j=256 = 2 k-tiles), attn quantized e4m3 x32.
    Then v' = wv_f avx (bf16), g = gelu(v'/32 + Bv) in fp8, out conv =
    fp8 DoubleRow over HIDE_C=256 with wo x8.  The residual is added on
    the PE (8x identity matmul into the out-conv PSUM group) so gelu2
    reads PSUM directly: out = gelu(o_ps/8 + Bo).
  * Image-pipelined emission: phase 1 of image n+1 (u convs, dots, c —
    pure PE + DVE casts) is interleaved into phase 2 of image n so the
    PE stream stays dense (HAM/pstate keeps the clock at 2.4 GHz).
  * PSUM (8 banks): v-pool 2x[128,1024] (v'/o rotate) + glue 2x[128,512]
    (u / avx halves) + dots [128,512] + small (c_ps/s_ps).
  * Engines: PE matmuls+residual; ACT exp + gelus on [128,1024] tiles
    (table swaps hidden via dummies); DVE u/avx casts, attn8, recip.
"""

import numpy as np

IN_C = 128
HIDE_C = 256
OUT_C = 128
WS = 7
SCALE = 0.125
EPS = 1e-5
B, H, W = 32, 112, 112
HW = H * W          # 12544
H1 = H // WS        # 16
W1 = W // WS        # 16
T = H1 * W1         # 256 windows
NP = WS * WS        # 49 positions
NCORES = 8
BPC = B // NCORES   # images per core

SM = 256.0          # M / h scale (exp applies 1/SM)
SA = 32.0           # attn fp8 scale (folded out in gelu1's scale)
SO = 8.0            # wo fp8 + residual-identity scale (gelu2 applies 1/SO)

F32 = np.float32


def build_bass_kernel(bpc=BPC):
    import concourse.bass as bass
    import concourse.tile as tile
    import concourse.mybir as mybir
    from concourse import bacc

    f32 = mybir.dt.float32
    bf16 = mybir.dt.bfloat16
    fp8 = mybir.dt.float8e4
    AF = mybir.ActivationFunctionType
    DR = mybir.MatmulPerfMode.DoubleRow
    ALU = mybir.AluOpType

    nc = bacc.Bacc("TRN2", target_bir_lowering=False)

    xw_d = nc.dram_tensor("x_win", [bpc, IN_C, NP * T], bf16,
                          kind="ExternalInput")
    xt_d = nc.dram_tensor("xT8", [bpc, 128, 2 * NP * 128], fp8,
                          kind="ExternalInput")
    m_d = nc.dram_tensor("m", [IN_C, IN_C], bf16, kind="ExternalInput")
    h_d = nc.dram_tensor("hcol", [IN_C, 1], bf16, kind="ExternalInput")
    i8_d = nc.dram_tensor("ident8", [128, 128], bf16, kind="ExternalInput")
    wvT_d = nc.dram_tensor("wvT", [IN_C, HIDE_C], bf16, kind="ExternalInput")
    wo_d = nc.dram_tensor("woP", [128, 2 * OUT_C], fp8, kind="ExternalInput")
    # packed per-partition fp32 bias columns: [Bv_lo, Bv_hi, Bo]
    bias_d = nc.dram_tensor("biases", [128, 3], f32, kind="ExternalInput")
    out_d = nc.dram_tensor("out", [bpc, OUT_C, NP * T], bf16,
                           kind="ExternalOutput")

    CHUNKS = [(k * 4, 4) for k in range(12)] + [(48, 1)]   # phase-2 granule
    UCH = [(k * 2, 2) for k in range(24)] + [(48, 1)]      # u-conv granule

    with tile.TileContext(nc) as tc:
        with (
            tc.tile_pool(name="singles", bufs=1) as singles,
            tc.tile_pool(name="xwin", bufs=2) as xw_pool,
            tc.tile_pool(name="xt8", bufs=2) as xt_pool,
            tc.tile_pool(name="u_sb", bufs=4) as u_pool,
            tc.tile_pool(name="attn", bufs=2) as attn_pool,
            tc.tile_pool(name="avx_sb", bufs=4) as avx_pool,
            tc.tile_pool(name="g_sb", bufs=2) as g_pool,
            tc.tile_pool(name="out_sb", bufs=3) as out_pool,
            tc.tile_pool(name="small_sb", bufs=2) as small_pool,
            tc.tile_pool(name="ps_v", bufs=2, space="PSUM") as ps_v,
            tc.tile_pool(name="ps_glue", bufs=2, space="PSUM") as ps_glue,
            tc.tile_pool(name="ps_dots", bufs=1, space="PSUM") as ps_dots,
            tc.tile_pool(name="ps_small", bufs=1, space="PSUM") as ps_small,
        ):
            # ---- weights / constants (resident) ----
            m_sb = singles.tile([128, IN_C], bf16)
            nc.sync.dma_start(out=m_sb, in_=m_d.ap())
            h_sb = singles.tile([128, 1], bf16)
            nc.sync.dma_start(out=h_sb, in_=h_d.ap())
            i8_sb = singles.tile([128, 128], bf16)
            nc.sync.dma_start(out=i8_sb, in_=i8_d.ap())
            wvT = singles.tile([128, HIDE_C], bf16)
            nc.sync.dma_start(out=wvT, in_=wvT_d.ap())
            woP = singles.tile([128, 2, OUT_C], fp8)
            nc.sync.dma_start(
                out=woP, in_=wo_d.ap().rearrange("p (k o) -> p k o", k=2))
            biases = singles.tile([128, 3], f32)
            nc.sync.dma_start(out=biases, in_=bias_d.ap())
            bv_ap = [biases[:, 0:1], biases[:, 1:2]]
            bo_ap = biases[:, 2:3]

            ones_mat = singles.tile([128, 128], bf16)
            nc.vector.memset(ones_mat, 1.0)
            ones_row = singles.tile([1, T], bf16)
            nc.vector.memset(ones_row, 1.0)

            def emit_p1(img):
                """Thunks for image img's loads + phase 1 + softmax.
                Returns (thunks, state); state['attn8'] etc filled as
                thunks run."""
                st = {}
                thunks = []

                def t_load():
                    x_win = xw_pool.tile([128, NP * T], bf16, tag="xw")
                    for dc in range(4):
                        s = dc * (NP * T // 4)
                        e = s + NP * T // 4
                        nc.sync.dma_start(out=x_win[:, s:e],
                                          in_=xw_d.ap()[img, :, s:e])
                    xT8 = xt_pool.tile([128, 2, NP, 128], fp8, tag="xt")
                    xt_src = xt_d.ap()[img].rearrange(
                        "p (k q c) -> p k q c", k=2, q=NP)
                    for dc in range(2):
                        nc.sync.dma_start(out=xT8[:, dc], in_=xt_src[:, dc])
                    # dummy exp pulls the exp table load off the chain
                    scratch = small_pool.tile([128, 1], f32, tag="scr")
                    nc.scalar.activation(scratch, biases[:, 0:1], AF.Exp)
                    dots_t = ps_dots.tile([128, 512], f32, tag="dots",
                                          name=f"dots{img}")
                    c_ps = ps_small.tile([1, T], f32, tag="psm",
                                         name=f"cps{img}")
                    st.update(x_win=x_win, xT8=xT8, scratch=scratch,
                              dots_t=dots_t, c_ps=c_ps, pend=[])
                thunks.append(t_load)

                def dots_mms(p0, npos, u_sb, first):
                    dots_t, x_win = st["dots_t"], st["x_win"]
                    for pi in range(npos):
                        for jh in (0, 1):
                            nc.tensor.matmul(
                                dots_t[:, jh * T:(jh + 1) * T],
                                lhsT=u_sb[:, pi * T + jh * 128:
                                          pi * T + jh * 128 + 128],
                                rhs=x_win[:, (p0 + pi) * T:(p0 + pi + 1) * T],
                                start=first and pi == 0 and jh == 0,
                                stop=False, skip_group_check=True)

                def mk_uchunk(ci, p0, npos):
                    def t():
                        x_win, c_ps = st["x_win"], st["c_ps"]
                        N = npos * T
                        u_ps = ps_glue.tile([128, 512], f32, tag="glue")
                        nc.tensor.matmul(u_ps[:, :N], lhsT=m_sb,
                                         rhs=x_win[:, p0 * T:p0 * T + N],
                                         start=True, stop=True)
                        u_sb = u_pool.tile([128, 512], bf16, tag="u")
                        nc.vector.tensor_copy(u_sb[:, :N], u_ps[:, :N])
                        # c run doubles as PE filler (no dep on the cast)
                        for pi in range(npos):
                            p = p0 + pi
                            nc.tensor.matmul(
                                c_ps, lhsT=h_sb,
                                rhs=x_win[:, p * T:(p + 1) * T],
                                start=p == 0, stop=p == NP - 1,
                                skip_group_check=True)
                        if len(st["pend"]) >= 2:
                            dots_mms(*st["pend"].pop(0))
                        st["pend"].append((p0, npos, u_sb, ci == 0))
                    return t

                for ci, (p0, npos) in enumerate(UCH):
                    thunks.append(mk_uchunk(ci, p0, npos))

                def t_softmax():
                    dots_t, c_ps = st["dots_t"], st["c_ps"]
                    for a in st["pend"]:
                        dots_mms(*a)
                    st["pend"] = []
                    c_sb = small_pool.tile([1, T], bf16, tag="csb")
                    nc.vector.tensor_copy(c_sb, c_ps)
                    for jh in (0, 1):
                        nc.tensor.matmul(
                            dots_t[:, jh * T:(jh + 1) * T],
                            lhsT=c_sb[:, jh * 128:jh * 128 + 128],
                            rhs=ones_row, start=False, stop=jh == 1,
                            skip_group_check=True)
                    attn_t = attn_pool.tile([128, 512], bf16, tag="at",
                                            name=f"attn_t{img}")
                    nc.scalar.activation(attn_t, dots_t, AF.Exp,
                                         scale=1.0 / SM)
                    s_ps = ps_small.tile([128, T], f32, tag="psm",
                                         name=f"ssum{img}")
                    for jc in (0, 1):
                        nc.tensor.matmul(s_ps, lhsT=ones_mat,
                                         rhs=attn_t[:, jc * T:(jc + 1) * T],
                                         start=jc == 0, stop=jc == 1)
                    r_sb = small_pool.tile([128, T], f32, tag="rsb")
                    nc.vector.reciprocal(r_sb, s_ps)
                    attn8 = attn_pool.tile([128, 2, T], fp8, tag="a8",
                                           name=f"attn8{img}")
                    for jc in (0, 1):
                        nc.vector.scalar_tensor_tensor(
                            attn8[:, jc], in0=attn_t[:, jc * T:(jc + 1) * T],
                            scalar=SA, op0=ALU.mult, in1=r_sb, op1=ALU.mult)
                    # dummy gelu pulls the gelu table load off the chain
                    nc.scalar.activation(st["scratch"], biases[:, 0:1],
                                         AF.Gelu)
                    st["attn8"] = attn8
                thunks.append(t_softmax)
                return thunks, st

            def emit_p2(img, st):
                """Thunks for image img's phase 2 (needs st from p1)."""
                def mk_chunk(p0, npos):
                    def t():
                        x_win, xT8, attn8 = (st["x_win"], st["xT8"],
                                             st["attn8"])
                        N = npos * T
                        # attention-average of x, in 2-position halves
                        avx_sb = avx_pool.tile([128, 1024], bf16, tag="avs")
                        for hb in range(0, npos, 2):
                            hp = min(2, npos - hb)
                            avx_ps = ps_glue.tile([128, 512], f32,
                                                  tag="glue")
                            for pi in range(hp):
                                nc.tensor.matmul(
                                    avx_ps[:, pi * T:(pi + 1) * T],
                                    lhsT=xT8[:, :, p0 + hb + pi, :],
                                    rhs=attn8, start=True, stop=True,
                                    perf_mode=DR)
                            nc.vector.tensor_copy(
                                avx_sb[:, hb * T:(hb + hp) * T],
                                avx_ps[:, :hp * T])
                        g_t = g_pool.tile([128, 2, 1024], fp8, tag="g")
                        for kc in (0, 1):
                            v_ps = ps_v.tile([128, 1024], f32, tag="v")
                            for hh in range(0, N, 512):
                                hn = min(512, N - hh)
                                nc.tensor.matmul(
                                    v_ps[:, hh:hh + hn],
                                    lhsT=wvT[:, kc * 128:(kc + 1) * 128],
                                    rhs=avx_sb[:, hh:hh + hn],
                                    start=True, stop=True)
                            nc.scalar.activation(g_t[:, kc, :N],
                                                 v_ps[:, :N], AF.Gelu,
                                                 bias=bv_ap[kc],
                                                 scale=1.0 / SA)
                        o_ps = ps_v.tile([128, 1024], f32, tag="v")
                        for hh in range(0, N, 512):
                            hn = min(512, N - hh)
                            nc.tensor.matmul(o_ps[:, hh:hh + hn], lhsT=woP,
                                             rhs=g_t[:, :, hh:hh + hn],
                                             start=True, stop=False,
                                             perf_mode=DR,
                                             skip_group_check=True)
                            # residual: o += 8*x on the PE; gelu2 scales 1/8
                            nc.tensor.matmul(
                                o_ps[:, hh:hh + hn], lhsT=i8_sb,
                                rhs=x_win[:, p0 * T + hh:p0 * T + hh + hn],
                                start=False, stop=True,
                                skip_group_check=True)
                        o_sb = out_pool.tile([128, 1024], bf16, tag="osb")
                        nc.scalar.activation(o_sb[:, :N], o_ps[:, :N],
                                             AF.Gelu, bias=bo_ap,
                                             scale=1.0 / SO)
                        nc.sync.dma_start(
                            out=out_d.ap()[img, :, p0 * T:p0 * T + N],
                            in_=o_sb[:, :N])
                    return t
                return [mk_chunk(p0, npos) for p0, npos in CHUNKS]

            # ---- image-pipelined drive loop ----
            p1, st = emit_p1(0)
            for t in p1:
                t()
            for img in range(bpc):
                p2 = emit_p2(img, st)
                if img + 1 < bpc:
                    nxt, nst = emit_p1(img + 1)
                else:
                    nxt, nst = [], None
                done = 0
                for k, t in enumerate(p2):
                    t()
                    want = (k + 1) * len(nxt) // len(p2)
                    while done < want:
                        nxt[done]()
                        done += 1
                while done < len(nxt):
                    nxt[done]()
                    done += 1
                st = nst

    nc.compile()
    return nc


def fold_params(wq, gq, bq, mq, vq, wk, gk, bk, mk, vk,
                wv, gv, bv, mv, vv, wo, bo, go, bbo, mo, vo):
    """Host-side BN/bias folding. Returns (M, h, wvT, woP, biases)."""
    import ml_dtypes
    bf16 = ml_dtypes.bfloat16
    e4m3 = ml_dtypes.float8_e4m3

    aq = gq / np.sqrt(vq + EPS)
    wq_f = (SCALE * aq)[:, None] * wq
    Bq = SCALE * (bq - aq * mq)

    ak = gk / np.sqrt(vk + EPS)
    wk_f = ak[:, None] * wk          # k bias drops (softmax shift invariance)

    M = (wk_f.T @ wq_f) * SM         # exp() applies 1/SM
    hv = (wk_f.T @ Bq) * SM

    av = gv / np.sqrt(vv + EPS)
    wv_f = av[:, None] * wv
    Bv = bv - av * mv                # applied inside the first gelu

    ao = go / np.sqrt(vo + EPS)
    wo_f = ao[:, None] * wo * SO     # gelu2 applies 1/SO
    Bo = ao * (bo - mo) + bbo        # conv bias + BN fold, inside last gelu

    # woP[dlo, k, o] = wo_f[o, k*128 + dlo]  (DoubleRow k-subtile pairs)
    woP = np.ascontiguousarray(
        wo_f.T.reshape(2, 128, OUT_C).transpose(1, 0, 2))

    biases = np.stack([Bv[:128], Bv[128:], Bo], axis=1).astype(F32)
    return (np.ascontiguousarray(M).astype(bf16),
            np.ascontiguousarray(hv[:, None]).astype(bf16),
            np.ascontiguousarray(wv_f.T).astype(bf16),
            np.clip(woP, -240, 240).astype(e4m3),
            biases)


_CACHED = {}


def _get_nc(bpc=BPC):
    if bpc not in _CACHED:
        _CACHED[bpc] = build_bass_kernel(bpc)
    return _CACHED[bpc]


def make_in_maps(inputs):
    import ml_dtypes
    bf16 = ml_dtypes.bfloat16
    e4m3 = ml_dtypes.float8_e4m3

    x = np.asarray(inputs["x"], F32)
    m, hv, wvT, woP, biases = fold_params(
        *[np.asarray(inputs[k], F32) for k in
          ("wq", "gq", "bq", "mq", "vq", "wk", "gk", "bk", "mk", "vk",
           "wv", "gv", "bv", "mv", "vv", "wo", "bo", "go", "bbo", "mo", "vo")]
    )
    ident8 = (SO * np.eye(128, dtype=F32)).astype(bf16)
    # window permute: [B, C, H, W] -> [B, C, p=(ws1 ws2), j=(h1 w1)]
    xwin = x.reshape(B, IN_C, H1, WS, W1, WS).transpose(0, 1, 3, 5, 2, 4)
    xwin = np.ascontiguousarray(xwin.reshape(B, IN_C, NP, T))
    xw_bf = xwin.reshape(B, IN_C, NP * T).astype(bf16)
    # token-major fp8: [B, j_local=128, (jc, p, ch)]
    xt = xwin.transpose(0, 3, 1, 2)          # [B, j, C, p]
    xt = xt.reshape(B, 2, 128, IN_C, NP).transpose(0, 2, 1, 4, 3)
    xt8 = np.clip(xt, -240, 240).astype(e4m3).reshape(B, 128, 2 * NP * 128)

    in_maps = []
    for c in range(NCORES):
        sl = slice(c * BPC, (c + 1) * BPC)
        in_maps.append({"x_win": np.ascontiguousarray(xw_bf[sl]),
                        "xT8": np.ascontiguousarray(xt8[sl]),
                        "m": m, "hcol": hv, "ident8": ident8, "wvT": wvT,
                        "woP": np.ascontiguousarray(
                            woP.reshape(128, 2 * OUT_C)),
                        "biases": biases})
    return in_maps


def kernel(**inputs):
    from concourse.bass_utils import run_bass_kernel_spmd

    in_maps = make_in_maps(inputs)
    nc = _get_nc(BPC)
    res = run_bass_kernel_spmd(nc, in_maps, list(range(NCORES)))
    outs = []
    for c in range(NCORES):
        o = np.asarray(res.results[c]["out"], dtype=F32)
        o = o.reshape(BPC, OUT_C, WS, WS, H1, W1).transpose(0, 1, 4, 2, 5, 3)
        outs.append(o.reshape(BPC, OUT_C, H, W))
    return np.concatenate(outs, axis=0)
